# revision 10
# baseline (speedup 1.0000x reference)
"""TRN2 Bass/Tile kernel for nn_DynamicGraphAttension (additive attention).

reference computation (per batch b):
    key_p   = key[b] @ W_key                      # [LFP]
    val_p   = value[b] @ W_value                  # [S, LFP]
    inter   = tanh(key_p + val_p)                 # [S, LFP]
    scores  = inter @ W_attn                      # [S]
    weights = softmax(scores)                     # [S]
    context = weights @ value[b]                  # [VD]
returns (context [B, VD], weights [B, S])

Sharding: data-parallel over batch B=32 across 8 cores (4 batches/core),
weights replicated (sharding_hint).

Per-core dataflow (single HBM pass over value, ~64 MiB f32):
  - value[b] loaded range-by-range (512 rows) as bf16 via SWDGE cast-DMA into
    natural tiles nat[p=s_lo, (a, v)] that stay resident for the whole batch.
  - xbar DMA-transpose (SBUF->SBUF, bf16) produces vT tiles [p=v, (a, s)] so the
    VD contraction can run on the PE (PE contracts along partitions).
  - projection: out = W_value_chunk.T @ vT -> val_p^T [l-chunk, s] in PSUM,
    accumulated over 8 v-chunks; bf16 = 1 cycle/row.
  - ACT: inter^T = tanh(val_p^T + key_p^T[l]) fused (per-partition bias),
    PSUM -> SBUF bf16.
  - scores: lhsT = W_attn chunk [128,1] (trivial LDWEIGHTS), rhs = inter^T
    -> psum [1, 512], accumulated over 4 l-chunks.
  - ACT: e = exp(scores) with fused accum_out partial sums (softmax has a hard
    bound |score| <= sum|W_attn| ~ 11.3, so no max-subtraction needed).
  - e rows are re-laid-out s-on-partitions with 4 tiny PE transposes/batch,
    normalized (1/sum broadcast via a rank-1 ones matmul), and used as lhsT
    [128,1] columns for the context matmuls against the resident natural tiles.
"""

import numpy as np
from contextlib import ExitStack

import concourse.bass as bass
import concourse.tile as tile
from concourse import bacc, mybir, masks
from concourse.bass_utils import run_bass_kernel_spmd

FP32 = mybir.dt.float32
BF16 = mybir.dt.bfloat16
ACT_TANH = mybir.ActivationFunctionType.Tanh
ACT_EXP = mybir.ActivationFunctionType.Exp

N_CORES = 8
B, S, KD, VD, LFP = 32, 4096, 1024, 1024, 512
BL = B // N_CORES  # batches per core


def build_program(nc, BL=BL, S=S, KD=KD, VD=VD, LFP=LFP):
    SC = 128            # s rows per chunk (one partition-block)
    RNG = 512           # s rows per range (one PSUM-bank free dim)
    CPR = RNG // SC     # 4 chunks per range
    NR = S // RNG       # ranges per batch
    NCH = S // SC       # chunks per batch
    NK = VD // 128      # v-chunks
    NJ = LFP // 128     # l-chunks
    NKD = KD // 128     # kd-chunks
    assert NR <= 128 and NCH <= 128

    key_d = nc.dram_tensor("key", [BL, KD], FP32, kind="ExternalInput").ap()
    val_d = nc.dram_tensor("value", [BL, S, VD], FP32, kind="ExternalInput").ap()
    wk_d = nc.dram_tensor("w_key", [KD, LFP], FP32, kind="ExternalInput").ap()
    wv_d = nc.dram_tensor("w_value", [VD, LFP], FP32, kind="ExternalInput").ap()
    wa_d = nc.dram_tensor("w_attn", [LFP], FP32, kind="ExternalInput").ap()
    ctx_d = nc.dram_tensor("out_ctx", [BL, VD], FP32, kind="ExternalOutput").ap()
    wts_d = nc.dram_tensor("out_wts", [BL, S], FP32, kind="ExternalOutput").ap()

    with tile.TileContext(nc) as tc, ExitStack() as ctx:
        consts = ctx.enter_context(tc.tile_pool(name="consts", bufs=1))
        nat_pool = ctx.enter_context(tc.tile_pool(name="nat", bufs=NR + NR // 2))
        vt_pool = ctx.enter_context(tc.tile_pool(name="vt", bufs=2))
        inter_pool = ctx.enter_context(tc.tile_pool(name="inter", bufs=2 * NJ))
        er_pool = ctx.enter_context(tc.tile_pool(name="er", bufs=2))
        small_sb = ctx.enter_context(tc.tile_pool(name="small", bufs=2))
        ps_val = ctx.enter_context(tc.tile_pool(name="ps_val", bufs=NJ, space="PSUM"))
        ps_small = ctx.enter_context(tc.tile_pool(name="ps_small", bufs=2, space="PSUM"))
        ps_eT_pool = ctx.enter_context(tc.tile_pool(name="ps_eT", bufs=1, space="PSUM"))
        ps_ctx = ctx.enter_context(tc.tile_pool(name="ps_ctx", bufs=1, space="PSUM"))

        # ---- constants / weights ----
        ident = consts.tile([128, 128], FP32, name="ident")
        masks.make_identity(nc, ident)
        ones128 = consts.tile([1, 128], FP32, name="ones128")
        nc.gpsimd.memset(ones128, 1.0)

        # W_value as bf16 chunks: wv_sb[p, k*LFP + l] = W_value[k*128+p, l]
        wv_sb = consts.tile([128, NK * LFP], BF16, name="wv_sb")
        nc.gpsimd.dma_start(
            wv_sb.rearrange("p (k l) -> p k l", k=NK),
            wv_d.rearrange("(k p) l -> p k l", p=128),
        )
        # W_key f32 chunks (key path computed in f32 for accuracy; it is tiny)
        wk_sb = consts.tile([128, NKD * LFP], FP32, name="wk_sb")
        nc.sync.dma_start(
            wk_sb.rearrange("p (k l) -> p k l", k=NKD),
            wk_d.rearrange("(k p) l -> p k l", p=128),
        )
        # W_attn as bf16 columns: wa_sb[p, j] = W_attn[j*128+p]
        wa_sb = consts.tile([128, NJ], BF16, name="wa_sb")
        nc.gpsimd.dma_start(wa_sb, wa_d.rearrange("(j p) -> p j", p=128))
        key_sb = consts.tile([BL, KD], FP32, name="key_sb")
        nc.sync.dma_start(key_sb, key_d)

        # ---- key projection: key_p^T columns keypT[p, j*BL+b] = key_p[b, j*128+p]
        ktT_sb = consts.tile([128, NKD * BL], FP32, name="ktT_sb")
        for k in range(NKD):
            pst = ps_small.tile([128, BL], FP32, name="ps_ktT", tag="ps_sm")
            nc.tensor.transpose(pst, key_sb[:, k * 128 : (k + 1) * 128], ident[:BL, :BL])
            nc.vector.tensor_copy(ktT_sb[:, k * BL : (k + 1) * BL], pst)
        psk = ps_small.tile([BL, LFP], FP32, name="ps_keyp", tag="ps_sm")
        for k in range(NKD):
            nc.tensor.matmul(
                psk,
                ktT_sb[:, k * BL : (k + 1) * BL],
                wk_sb[:, k * LFP : (k + 1) * LFP],
                start=(k == 0),
                stop=(k == NKD - 1),
            )
        kp_sb = consts.tile([BL, LFP], FP32, name="kp_sb")
        nc.vector.tensor_copy(kp_sb, psk)
        keypT_sb = consts.tile([128, NJ * BL], FP32, name="keypT_sb")
        for j in range(NJ):
            pst = ps_small.tile([128, BL], FP32, name="ps_ktT", tag="ps_sm")
            nc.tensor.transpose(pst, kp_sb[:, j * 128 : (j + 1) * 128], ident[:BL, :BL])
            nc.vector.tensor_copy(keypT_sb[:, j * BL : (j + 1) * BL], pst)

        # ---- main loop over batches ----
        for b in range(BL):
            nat_tiles = []
            partials = small_sb.tile([1, NR], FP32, name="partials")
            # eT[p, c] = exp(scores)[c*128 + p], filled column-wise by PE transposes
            ps_eT = ps_eT_pool.tile([128, NCH], FP32, name="ps_eT")
            for r in range(NR):
                # load one range of value as bf16 (cast during DMA):
                # natr[p, a*VD + v] = value[b, r*RNG + a*128 + p, v]
                natr = nat_pool.tile([128, CPR * VD], BF16, name="natr", tag="nat")
                nc.gpsimd.dma_start(
                    natr.rearrange("p (a v) -> p a v", a=CPR),
                    val_d[b, r * RNG : (r + 1) * RNG, :].rearrange(
                        "(a p) v -> p a v", p=128
                    ),
                )
                nat_tiles.append(natr)
                # transpose to vtr[p, a*VD + k*128 + s] = value^T[v=k*128+p, s]
                vtr = vt_pool.tile([128, CPR * VD], BF16, name="vtr", tag="vt")
                for a in range(CPR):
                    nc.sync.dma_start(
                        out=vtr[:, a * VD : (a + 1) * VD].rearrange(
                            "p (k s) -> p k s", k=NK
                        ),
                        in_=natr[:, a * VD : (a + 1) * VD],
                        transpose=True,
                    )
                # projection: val_p^T per l-chunk, accumulated over v-chunks
                vtr_k = vtr.rearrange("p (a k s) -> p k a s", a=CPR, k=NK)
                inters = []
                for j in range(NJ):
                    psv = ps_val.tile([128, RNG], FP32, name="ps_valp", tag="psv")
                    for k in range(NK):
                        nc.tensor.matmul(
                            psv,
                            wv_sb[:, k * LFP + j * 128 : k * LFP + (j + 1) * 128],
                            vtr_k[:, k],
                            start=(k == 0),
                            stop=(k == NK - 1),
                        )
                    itj = inter_pool.tile([128, RNG], BF16, name="interT", tag="it")
                    nc.scalar.activation(
                        itj, psv, ACT_TANH,
                        bias=keypT_sb[:, j * BL + b : j * BL + b + 1],
                    )
                    inters.append(itj)
                # scores for this range: [1, RNG], accumulated over l-chunks
                ps_sc = ps_small.tile([1, RNG], FP32, name="ps_sc", tag="ps_sm")
                for j in range(NJ):
                    nc.tensor.matmul(
                        ps_sc, wa_sb[:, j : j + 1], inters[j],
                        start=(j == 0), stop=(j == NJ - 1),
                    )
                # e = exp(scores) with fused partial sum
                er = er_pool.tile([1, RNG], FP32, name="er", tag="er")
                nc.scalar.activation(
                    er, ps_sc, ACT_EXP, accum_out=partials[0:1, r : r + 1],
                )
                # scatter e to s-on-partitions columns: eT[p, c] = e[c*128+p]
                for i in range(CPR):
                    c = r * CPR + i
                    nc.tensor.transpose(
                        ps_eT[:, c : c + 1],
                        er[0:1, i * 128 : (i + 1) * 128],
                        ident[:1, :1],
                    )

            # softmax normalizer
            sum_sb = small_sb.tile([1, 1], FP32, name="sum_sb")
            nc.vector.reduce_sum(sum_sb, partials, axis=mybir.AxisListType.X)
            recip = small_sb.tile([1, 1], FP32, name="recip")
            nc.vector.reciprocal(recip, sum_sb)
            # broadcast 1/sum to all partitions via rank-1 matmul
            ps_bc = ps_small.tile([128, 1], FP32, name="ps_bc", tag="ps_sm")
            nc.tensor.matmul(ps_bc, ones128, recip, start=True, stop=True)
            recip128 = small_sb.tile([128, 1], FP32, name="recip128")
            nc.vector.tensor_copy(recip128, ps_bc)

            eT_f32 = small_sb.tile([128, NCH], FP32, name="eT_f32")
            nc.vector.tensor_copy(eT_f32, ps_eT)
            wT_f32 = small_sb.tile([128, NCH], FP32, name="wT_f32")
            nc.vector.tensor_scalar_mul(wT_f32, eT_f32, recip128)
            wT_bf16 = small_sb.tile([128, NCH], BF16, name="wT_bf16")
            nc.vector.tensor_copy(wT_bf16, wT_f32)

            # weights output: transpose back to natural order and store
            ps_wn = ps_small.tile([NCH, 128], FP32, name="ps_wn", tag="ps_sm")
            nc.tensor.transpose(ps_wn, wT_f32, ident)
            wnat = small_sb.tile([NCH, 128], FP32, name="wnat")
            nc.vector.tensor_copy(wnat, ps_wn)
            nc.sync.dma_start(wts_d[b].rearrange("(c s) -> c s", s=128), wnat)

            # context: ctx[v] = sum_s w[s] * value[b, s, v] over resident nat tiles
            ctx_sb = small_sb.tile([1, VD], FP32, name="ctx_sb")
            for h in range(VD // 512):
                ps_c = ps_ctx.tile([1, 512], FP32, name="ps_c")
                for c in range(NCH):
                    r, a = divmod(c, CPR)
                    nc.tensor.matmul(
                        ps_c,
                        wT_bf16[:, c : c + 1],
                        nat_tiles[r][:, a * VD + h * 512 : a * VD + (h + 1) * 512],
                        start=(c == 0),
                        stop=(c == NCH - 1),
                    )
                nc.vector.tensor_copy(ctx_sb[0:1, h * 512 : (h + 1) * 512], ps_c)
            nc.sync.dma_start(ctx_d[b : b + 1, :], ctx_sb)

    nc.compile()
    return nc


_CACHE = {}


def _get_program():
    if "nc" not in _CACHE:
        nc = bacc.Bacc(
            "TRN2", target_bir_lowering=False, debug=False, num_devices=N_CORES
        )
        build_program(nc)
        _CACHE["nc"] = nc
    return _CACHE["nc"]


def kernel(key_tensor, value_tensor, W_key, W_value, W_attn, _trace=False):
    nc = _get_program()
    in_maps = []
    for c in range(N_CORES):
        sl = slice(c * BL, (c + 1) * BL)
        in_maps.append(
            {
                "key": np.ascontiguousarray(key_tensor[sl], dtype=np.float32),
                "value": np.ascontiguousarray(value_tensor[sl], dtype=np.float32),
                "w_key": np.ascontiguousarray(W_key, dtype=np.float32),
                "w_value": np.ascontiguousarray(W_value, dtype=np.float32),
                "w_attn": np.ascontiguousarray(W_attn, dtype=np.float32),
            }
        )
    res = run_bass_kernel_spmd(
        nc, in_maps, core_ids=list(range(N_CORES)), trace=_trace
    )
    _CACHE["last_results"] = res
    context = np.concatenate([r["out_ctx"] for r in res.results], axis=0)
    weights = np.concatenate([r["out_wts"] for r in res.results], axis=0)
    return context, weights
